# revision 19
# baseline (speedup 1.0000x reference)
"""Trainium2 Bass kernel for nn_DeepSpeedMLP (fused LN + MLP + residual).

Math (per reference):
    pre = input + residual + bias            # residual stream
    ln  = layernorm(pre) * attn_nw + attn_nb
    h   = gelu(ln @ inter_w + inter_b)       # exact (erf) gelu
    out = h @ output_w + output_b + pre

Distribution (8 cores, Megatron tensor-parallel over the intermediate dim):
  - inter_w/inter_b column-sharded (2048 cols/core), output_w row-sharded.
  - LayerNorm is token-sharded (512 tokens/core) then AllGather'd (fp16).
  - fc2 partial outputs are ReduceScatter'd in 16 chunks (token-tile x
    H-half), overlapped with fc2 compute; each core finishes its own
    1/8 of the tokens (residual add) and the host concatenates.

On-device dtypes: matmuls run fp16 x fp16 -> fp32 PSUM (1 cycle/row on PE,
~2^-11 operand rounding). LN statistics, residual stream, fc2 partials and
the collectives' reduction all stay fp32.

attn_nw/attn_nb are folded on the host into inter_w/inter_b (exact algebra:
ln_norm*nw + nb feeding fc1 == ln_norm @ (nw*W1) + (b1 + nb@W1)). The
per-H bias vectors (bias, output_b) are likewise folded into the token
inputs on the host, so the device only does LN stats + normalize, the two
GEMMs, gelu, the collectives and the residual accumulation.
"""

import numpy as np

import concourse.bass as bass
import concourse.tile as tile
from concourse import bacc, mybir
from concourse import bass_utils

# Problem shape (hardcoded per spec nn_DeepSpeedMLP_31035433681240)
B, S, H, I = 2, 2048, 4096, 16384
TOK = B * S            # 4096 tokens
NCORES = 8
TSH = TOK // NCORES    # 512 tokens per core (LN shard / output shard)
ISH = I // NCORES      # 2048 intermediate cols per core
P = 128
EPS = 1e-5
F32 = mybir.dt.float32
F16 = mybir.dt.float16
RG = [list(range(NCORES))]
ACT_FUNC = mybir.ActivationFunctionType.Gelu  # sim_test patches to Relu

_CACHED = {}


def _build(TOK=TOK, H=H, I=I):
    TSH = TOK // NCORES
    ISH = I // NCORES
    HC = H // P                    # fc1 contraction chunks
    IC = ISH // P                  # fc2 contraction chunks
    NT = TOK // P                  # token blocks
    STRIP = TSH // NCORES          # rows per RS-output strip
    RPC = TSH // P                 # token blocks per ReduceScatter chunk
    NCH = NT // RPC                # token chunks for ReduceScatter (=NCORES)
    N1 = (ISH // 2) // 512         # fc1 free-dim chunks per half
    N2 = (H // 2) // 512           # fc2 free-dim chunks per half
    assert (ISH // 2) % 512 == 0 and (H // 2) % 512 == 0 and H % 512 == 0

    nc = bacc.Bacc("TRN2", target_bir_lowering=False, debug=False,
                   num_devices=NCORES)

    def din(name, shape):
        return nc.dram_tensor(name, list(shape), F32, kind="ExternalInput").ap()

    x_ln = din("x_ln", (TSH, H))   # input block + bias (host-folded)
    r_ln = din("r_ln", (TSH, H))
    x_fin = din("x_fin", (TSH, H))  # input strips + bias + output_b
    r_fin = din("r_fin", (TSH, H))
    w1 = din("w1", (H, ISH))       # attn_nw folded in on host
    b1 = din("b1", (ISH,))        # attn_nb@W1 folded in on host
    w2 = din("w2", (ISH, H))
    out = nc.dram_tensor("out", [TSH, H], F32, kind="ExternalOutput").ap()

    def bcast(vec_ap, n):
        # [n] DRAM vector -> partition-broadcast [P, n] AP (stride 0)
        return bass.AP(tensor=vec_ap.tensor, offset=vec_ap.offset,
                       ap=[[0, P], *vec_ap.ap])

    with tile.TileContext(nc) as tc:
        with tc.tile_pool(name="dram", bufs=1, space="DRAM") as dram, \
             tc.tile_pool(name="wpool", bufs=2) as wpool, \
             tc.tile_pool(name="cpool", bufs=1) as cpool:

            ln_sh = dram.tile([TSH, H], F16)
            ln_all = dram.tile([TOK, H], F16, addr_space="Shared")
            h_dram = dram.tile([TOK, ISH], F16)
            pc = [dram.tile([TSH, H // 2], F32, name=f"pc{t}")
                  for t in range(2 * NCH)]
            ro = [dram.tile([STRIP, H // 2], F32, name=f"ro{t}")
                  for t in range(2 * NCH)]

            w1v = w1.rearrange("(hc p) i -> p hc i", p=P)
            w2v = w2.rearrange("(ic p) h -> p ic h", p=P)

            def load_w1(s):
                t = wpool.tile([P, HC, ISH // 2], F16, tag="W",
                               name=f"w1t{s}")
                q = max(HC // 4, 1)
                for o in range(0, HC, q):
                    nc.gpsimd.dma_start(
                        t[:, o:o + q, :],
                        w1v[:, o:o + q, (ISH // 2) * s:(ISH // 2) * (s + 1)])
                return t

            def load_w2(s):
                t = wpool.tile([P, IC, H // 2], F16, tag="W",
                               name=f"w2t{s}")
                q = max(IC // 4, 1)
                for o in range(0, IC, q):
                    nc.gpsimd.dma_start(
                        t[:, o:o + q, :],
                        w2v[:, o:o + q, (H // 2) * s:(H // 2) * (s + 1)])
                return t

            # fc1 weights: both halves cast-DMA'd (f32->f16) up front;
            # they overlap the LN phase on the SWDGE queues.
            w1t = [load_w1(0), load_w1(1)]

            # ---------------- LN phase (this core's TSH tokens) -------------
            with tc.tile_pool(name="lnp", bufs=2) as lnp, \
                 tc.tile_pool(name="lnc", bufs=1) as lnc:
                eps_t = lnc.tile([P, 1], F32)
                nc.vector.memset(eps_t[:], EPS)
                for t in range(TSH // P):
                    xt = lnp.tile([P, H], F32, tag="xt", bufs=1)
                    nc.sync.dma_start(xt[:], x_ln[P * t:P * (t + 1), :])
                    # accum-DMA rows capped at 2048 f32 elements (CCE limit)
                    for o in range(0, H, 2048):
                        nc.gpsimd.dma_start(
                            xt[:, o:o + 2048],
                            r_ln[P * t:P * (t + 1), o:o + 2048],
                            accum_op=mybir.AluOpType.add)
                    # xt is now pre = input + residual + bias
                    stats = lnp.tile([P, H // 512, 6], F32, tag="st")
                    for c in range(H // 512):
                        nc.vector.bn_stats(stats[:, c, :],
                                           xt[:, 512 * c:512 * (c + 1)])
                    mv = lnp.tile([P, 2], F32, tag="mv")
                    nc.vector.bn_aggr(mv[:], stats[:])
                    std = lnp.tile([P, 1], F32, tag="sd")
                    nc.scalar.activation(std[:], mv[:, 1:2],
                                         mybir.ActivationFunctionType.Sqrt,
                                         bias=eps_t[:], scale=1.0)
                    nc.vector.reciprocal(std[:], std[:])
                    ln16 = lnp.tile([P, H], F16, tag="ln16", bufs=1)
                    nc.vector.tensor_scalar(
                        out=ln16[:], in0=xt[:], scalar1=mv[:, 0:1],
                        scalar2=std[:], op0=mybir.AluOpType.subtract,
                        op1=mybir.AluOpType.mult)
                    nc.sync.dma_start(ln_sh[P * t:P * (t + 1), :], ln16[:])

            nc.gpsimd.collective_compute(
                "AllGather", mybir.AluOpType.bypass, replica_groups=RG,
                ins=[ln_sh.opt()], outs=[ln_all.opt()])

            # ---------------- fc1: h = gelu(ln @ w1 + b1) -------------------
            b1_bc = cpool.tile([P, ISH], F32)
            nc.gpsimd.dma_start(out=b1_bc[:], in_=bcast(b1, ISH))

            with tc.tile_pool(name="f1p", bufs=2) as f1p, \
                 tc.tile_pool(name="psp1", bufs=2, space="PSUM") as psp1:
                for s in range(2):
                    for pb in range(NT):
                        lnT = f1p.tile([P, HC, P], F16, tag="lnT")
                        # split transposes to <=2048 free-dim (HW quirk)
                        for o in range(0, HC, 16):
                            nc.sync.dma_start_transpose(
                                lnT[:, o:o + 16, :],
                                ln_all[P * pb:P * (pb + 1),
                                       P * o:P * (o + 16)])
                        ps = psp1.tile([P, N1, 512], F32, tag="ps",
                                       name=f"ps1_{s}_{pb}")
                        for hc in range(HC):
                            for i_n in range(N1):
                                nc.tensor.matmul(
                                    ps[:, i_n, :],
                                    lhsT=lnT[:, hc, :],
                                    rhs=w1t[s][:, hc, 512 * i_n:512 * (i_n + 1)],
                                    start=(hc == 0), stop=(hc == HC - 1))
                        ht = f1p.tile([P, ISH // 2], F16, tag="ht")
                        tmp = f1p.tile([P, ISH // 2], F32, tag="tmp")
                        nc.vector.tensor_add(
                            tmp[:], ps.rearrange("p n w -> p (n w)"),
                            b1_bc[:, (ISH // 2) * s:(ISH // 2) * (s + 1)])
                        nc.scalar.activation(ht[:], tmp[:], ACT_FUNC)
                        nc.sync.dma_start(
                            h_dram[P * pb:P * (pb + 1),
                                   (ISH // 2) * s:(ISH // 2) * (s + 1)], ht[:])
                    if s == 0:
                        # w2 half 0 casts during fc1's second half
                        w2t0 = load_w2(0)

            # ---------------- fc2 + ReduceScatter + residual ---------------
            with tc.tile_pool(name="f2p", bufs=2) as f2p, \
                 tc.tile_pool(name="psp2", bufs=2, space="PSUM") as psp2:
                for s in range(2):
                    w2t = w2t0 if s == 0 else load_w2(1)
                    for pb in range(NT):
                        hT = f2p.tile([P, IC, P], F16, tag="hT")
                        nc.sync.dma_start_transpose(
                            hT[:], h_dram[P * pb:P * (pb + 1), :])
                        ps2 = psp2.tile([P, N2, 512], F32, tag="ps",
                                        name=f"ps2_{s}_{pb}")
                        for ic in range(IC):
                            for n in range(N2):
                                nc.tensor.matmul(
                                    ps2[:, n, :],
                                    lhsT=hT[:, ic, :],
                                    rhs=w2t[:, ic, 512 * n:512 * (n + 1)],
                                    start=(ic == 0), stop=(ic == IC - 1))
                        ot = f2p.tile([P, H // 2], F32, tag="ot")
                        nc.vector.tensor_copy(
                            ot[:], ps2.rearrange("p n w -> p (n w)"))
                        Tp, rr = divmod(pb, RPC)
                        t = Tp * 2 + s
                        nc.sync.dma_start(pc[t][P * rr:P * (rr + 1), :], ot[:])
                        if rr == RPC - 1:
                            nc.gpsimd.collective_compute(
                                "ReduceScatter", mybir.AluOpType.add,
                                replica_groups=RG,
                                ins=[pc[t].opt()], outs=[ro[t].opt()])
                            ft = f2p.tile([STRIP, H // 2], F32, tag="ft", bufs=1)
                            nc.sync.dma_start(ft[:], ro[t][:])
                            nc.gpsimd.dma_start(
                                ft[:],
                                x_fin[STRIP * Tp:STRIP * (Tp + 1),
                                      (H // 2) * s:(H // 2) * (s + 1)],
                                accum_op=mybir.AluOpType.add)
                            nc.gpsimd.dma_start(
                                ft[:],
                                r_fin[STRIP * Tp:STRIP * (Tp + 1),
                                      (H // 2) * s:(H // 2) * (s + 1)],
                                accum_op=mybir.AluOpType.add)
                            nc.sync.dma_start(
                                out[STRIP * Tp:STRIP * (Tp + 1),
                                    (H // 2) * s:(H // 2) * (s + 1)], ft[:])

    nc.compile()
    return nc


def get_nc():
    if "nc" not in _CACHED:
        _CACHED["nc"] = _build()
    return _CACHED["nc"]


def make_in_maps(input, residual, bias, attn_nw, attn_nb, inter_w, inter_b,
                 output_w, output_b, TOK=TOK, H=H, I=I):
    TSH = TOK // NCORES
    ISH = I // NCORES
    STRIP = TSH // NCORES

    cbias = bias + output_b
    nw_triv = bool(np.all(attn_nw == 1.0))
    nb_triv = bool(np.all(attn_nb == 0.0))

    # host-folded bias adds (broadcast along tokens)
    x_biased = input + bias            # LN path: pre = x_biased + residual
    x_final = input + cbias            # final path: out = rs + x_final + r
    # strided strips: core j's final tokens are {Tp*TSH + STRIP*j + t}
    x_strips = x_final.reshape(NCORES, NCORES, STRIP, H)   # [Tp, j, t, H]
    r_strips = residual.reshape(NCORES, NCORES, STRIP, H)

    in_maps = []
    for j in range(NCORES):
        sl = slice(ISH * j, ISH * (j + 1))
        w1j = inter_w[:, sl]
        if not nw_triv:
            w1j = w1j * attn_nw[:, None]
        b1j = inter_b[sl]
        if not nb_triv:
            b1j = b1j + attn_nb @ inter_w[:, sl]
        in_maps.append({
            "x_ln": np.ascontiguousarray(x_biased[TSH * j:TSH * (j + 1)]),
            "r_ln": np.ascontiguousarray(residual[TSH * j:TSH * (j + 1)]),
            "x_fin": np.ascontiguousarray(x_strips[:, j].reshape(TSH, H)),
            "r_fin": np.ascontiguousarray(r_strips[:, j].reshape(TSH, H)),
            "w1": np.ascontiguousarray(w1j, dtype=np.float32),
            "b1": np.ascontiguousarray(b1j, dtype=np.float32),
            "w2": np.ascontiguousarray(output_w[sl]),
        })
    return in_maps


def assemble(shards_list, TOK=TOK, H=H):
    TSH = TOK // NCORES
    STRIP = TSH // NCORES
    shards = np.stack(shards_list)
    # shards[j, Tp*STRIP+t] = token Tp*TSH + STRIP*j + t
    full = shards.reshape(NCORES, NCORES, STRIP, H).transpose(1, 0, 2, 3)
    return np.ascontiguousarray(full.reshape(TOK, H))


def kernel(input, residual, residual_norm, bias, weight,
           attn_nw, attn_nb, inter_w, inter_b, output_w, output_b,
           **_ignored):
    input = np.asarray(input, dtype=np.float32).reshape(TOK, H)
    residual = np.asarray(residual, dtype=np.float32).reshape(TOK, H)
    bias = np.asarray(bias, dtype=np.float32)
    attn_nw = np.asarray(attn_nw, dtype=np.float32)
    attn_nb = np.asarray(attn_nb, dtype=np.float32)
    inter_w = np.asarray(inter_w, dtype=np.float32)
    inter_b = np.asarray(inter_b, dtype=np.float32)
    output_w = np.asarray(output_w, dtype=np.float32)
    output_b = np.asarray(output_b, dtype=np.float32)

    in_maps = make_in_maps(input, residual, bias, attn_nw, attn_nb,
                           inter_w, inter_b, output_w, output_b)
    nc = get_nc()
    res = bass_utils.run_bass_kernel_spmd(
        nc, in_maps, core_ids=list(range(NCORES)),
        **_CACHED.get("run_kwargs", {}))
    _CACHED["last_result"] = res

    full = assemble([res.results[j]["out"] for j in range(NCORES)])
    return full.reshape(B, S, H)
